# revision 60
# baseline (speedup 1.0000x reference)
"""Trainium2 Bass kernel for the gaussian-mixture ray decoder.

Math: quad[n,m] = (pos_n - mu_m)^T Sigma_inv_m (pos_n - mu_m) expands to
F[n,:16] @ C[m,:16] with F = [pairwise products(10), pos(4), 1, pad] and
C = [Sigma_inv entries (off-diag doubled), -2 Sigma_inv mu, mu^T Sigma_inv mu, pad].
Then out = sigmoid(sum_m exp(-0.5 quad[n,m]) * labels[m]).

Device layout (per core, N sharded 8 x 4096 rays):
  - featT [16, 4096] (F^T shard), coefT [16, 1024] (C^T), labels_p [128, 8]
    (labels[mc*128+p] at [p, mc]) live in SBUF.
  - for each ray group g (4 x 1024 rays) x gaussian chunk mc (8 x 128):
      PE  : quad psum tile [128 gauss, 1024 rays] via 2 float32r matmuls
      ACT : e = exp(-0.5 quad)  (one [128,1024] instr, PSUM -> SBUF)
      PE  : acc[1,1024] += labels_chunk^T @ e  (f32 PSUM accumulation over mc)
  - sigmoid(x) = 0.5*tanh(0.5 x) + 0.5 (tanh shares the exp ACT table set),
    affine on DVE, DMA out.
"""

import sys
from contextlib import ExitStack

import numpy as np

sys.path.insert(0, "/opt/trn_rl_repo")

import concourse.bacc as bacc
import concourse.tile as tile
from concourse import mybir, bass_utils

N, M, D = 32768, 1024, 4
N_CORES = 8
NC_RAYS = N // N_CORES          # 4096 rays per core
RG = 1024                       # rays per group
N_GROUPS = NC_RAYS // RG        # 4
MC = 128                        # gaussians per chunk
M_CHUNKS = M // MC              # 8
K = 16                          # feature/coefficient length (15 used + 1 pad)

F32 = mybir.dt.float32
F32R = mybir.dt.float32r

_CACHE = {}
LAST_RESULTS = None  # BassKernelResults of the most recent run (for test harness)


def _build_bass():
    nc = bacc.Bacc("TRN2", target_bir_lowering=False, debug=False)

    featT_d = nc.dram_tensor("featT", [K, NC_RAYS], F32R, kind="ExternalInput").ap()
    coefT_d = nc.dram_tensor("coefT", [K, M], F32R, kind="ExternalInput").ap()
    labels_d = nc.dram_tensor("labels_p", [MC, M_CHUNKS], F32R, kind="ExternalInput").ap()
    out_d = nc.dram_tensor("out", [N_GROUPS, RG], F32, kind="ExternalOutput").ap()

    with tile.TileContext(nc) as tc:
        with ExitStack() as ctx:
            const_pool = ctx.enter_context(tc.tile_pool(name="const", bufs=1))
            e_pool = ctx.enter_context(tc.tile_pool(name="e", bufs=8))
            fin_pool = ctx.enter_context(tc.tile_pool(name="fin", bufs=1))
            q_pool = ctx.enter_context(tc.tile_pool(name="q", bufs=3, space="PSUM"))
            acc_pool = ctx.enter_context(tc.tile_pool(name="acc", bufs=1, space="PSUM"))

            featT = const_pool.tile([K, NC_RAYS], F32R)
            coefT = const_pool.tile([K, M], F32R)
            labels = const_pool.tile([MC, M_CHUNKS], F32R)

            # coefT + group-0 featT gate the first matmul: run them on separate
            # rings in parallel. Only ONE descriptor goes on the ACT ring
            # (prep would delay the ACT table load); the rest ride the SP ring.
            nc.scalar.dma_start(featT[:, 0:RG], featT_d[:, 0:RG])
            nc.sync.dma_start(coefT[:], coefT_d[:])
            nc.sync.dma_start(labels[:], labels_d[:])
            for g in range(1, N_GROUPS):
                nc.sync.dma_start(
                    featT[:, g * RG:(g + 1) * RG], featT_d[:, g * RG:(g + 1) * RG]
                )

            # PE warmup: the HAM clock gate holds PE at half rate until ~3.4us
            # of sustained activity; burn the whole input-DMA wait (~3us) on
            # tiny matmuls over zeroed scratch so the real matmul stream runs
            # at full rate from its first instruction.
            wsb = const_pool.tile([K, 640], mybir.dt.bfloat16)
            nc.gpsimd.memset(wsb[:], 0.0)
            for w in range(32):
                wq = q_pool.tile([MC, RG], F32, tag="q")
                nc.tensor.matmul(
                    wq[:, 0:64],
                    lhsT=wsb[:, 0:MC],
                    rhs=wsb[:, 512:576],
                    start=True,
                    stop=True,
                )

            # per-group sums staged into 32-aligned rows of one SBUF tile so
            # the sigmoid tail is one batched ACT/DVE pass + one strided DMA
            sums = fin_pool.tile([MC, RG], F32)

            last_acc = None
            for g in range(N_GROUPS):
                acc = acc_pool.tile([1, RG], F32)
                last_g = g == N_GROUPS - 1
                for mc in range(M_CHUNKS):
                    q = q_pool.tile([MC, RG], F32)
                    lhs_c = coefT[:, mc * MC:(mc + 1) * MC]
                    for h in range(2):
                        rays = slice(g * RG + h * 512, g * RG + (h + 1) * 512)
                        nc.tensor.matmul(
                            q[:, h * 512:(h + 1) * 512],
                            lhsT=lhs_c,
                            rhs=featT[:, rays],
                            start=True,
                            stop=True,
                        )
                    e = e_pool.tile([MC, RG], F32R)
                    nc.scalar.activation(
                        e[:], q[:], mybir.ActivationFunctionType.Exp, scale=-0.5
                    )
                    lhs_l = labels[:, mc:mc + 1]
                    for h in range(2):
                        nc.tensor.matmul(
                            acc[:, h * 512:(h + 1) * 512],
                            lhsT=lhs_l,
                            rhs=e[:, h * 512:(h + 1) * 512],
                            start=(mc == 0),
                            stop=(mc == M_CHUNKS - 1),
                        )
                if last_g:
                    last_acc = acc
                else:
                    # stage groups 0..2 on the idle DVE into 32-aligned rows;
                    # their sigmoid runs for free in the post-stream ACT gap
                    for h in range(2):
                        cols = slice(h * 512, (h + 1) * 512)
                        nc.vector.tensor_scalar_add(
                            sums[32 * g:32 * g + 1, cols], acc[0:1, cols], 0.0
                        )

            # sigmoid(x) = 0.5 tanh(0.5 x) + 0.5 (Tanh shares the exp table
            # set). Groups 0..2: one batched pass over the staged rows — ACT
            # is idle right after the last exp, so this hides completely.
            # Last group: tanh reads its PSUM accumulator directly (no DVE
            # staging on the critical path), in column halves.
            rows_a = 32 * (N_GROUPS - 2) + 1  # covers staged rows 0..64
            th = fin_pool.tile([MC, RG], F32)
            res = fin_pool.tile([MC, RG], F32)
            nc.scalar.activation(
                th[:rows_a, :], sums[:rows_a, :],
                mybir.ActivationFunctionType.Tanh, scale=0.5,
            )
            nc.vector.tensor_scalar(
                res[:rows_a, :], th[:rows_a, :], 0.5, 0.5,
                mybir.AluOpType.mult, mybir.AluOpType.add,
            )
            nc.sync.dma_start(
                out_d[0:N_GROUPS - 1, :], res[0:rows_a:32, :]
            )
            lrow = 32 * (N_GROUPS - 1)
            for h in range(2):
                cols = slice(h * 512, (h + 1) * 512)
                nc.scalar.activation(
                    th[lrow:lrow + 1, cols], last_acc[0:1, cols],
                    mybir.ActivationFunctionType.Tanh, scale=0.5,
                )
                nc.vector.tensor_scalar(
                    res[lrow:lrow + 1, cols], th[lrow:lrow + 1, cols], 0.5, 0.5,
                    mybir.AluOpType.mult, mybir.AluOpType.add,
                )
            nc.sync.dma_start(
                out_d[N_GROUPS - 1:N_GROUPS, :], res[lrow:lrow + 1, :]
            )

    nc.compile()
    return nc


def _host_prepare(origins, directions, means, covariances, labels_embedding):
    pos = np.concatenate(
        [origins.astype(np.float64), directions.astype(np.float64)], axis=1
    )  # [N,4]
    S = np.linalg.inv(covariances.astype(np.float64))  # [M,4,4]
    mu = means.astype(np.float64)

    pairs = [(i, j) for i in range(D) for j in range(i, D)]  # 10
    F = np.zeros((N, K), dtype=np.float64)
    for k, (i, j) in enumerate(pairs):
        F[:, k] = pos[:, i] * pos[:, j]
    F[:, 10:14] = pos
    F[:, 14] = 1.0

    C = np.zeros((M, K), dtype=np.float64)
    for k, (i, j) in enumerate(pairs):
        C[:, k] = S[:, i, j] * (1.0 if i == j else 2.0)
    C[:, 10:14] = -2.0 * np.einsum("mij,mj->mi", S, mu)
    C[:, 14] = np.einsum("mi,mij,mj->m", mu, S, mu)

    # note: uploading full-f32 values as float32r measures MORE accurate on HW
    # than pre-rounding to 10-bit TF32 (1.5e-3 vs 4.1e-3 final rel err) — the
    # PE's f32r path keeps more effective mantissa than the TF32 model.
    coefT = np.ascontiguousarray(C.T.astype(np.float32))  # [16, M]
    labels_p = np.ascontiguousarray(
        labels_embedding.astype(np.float32).reshape(M_CHUNKS, MC).T
    )  # [128, 8]
    featT_shards = [
        np.ascontiguousarray(F[c * NC_RAYS:(c + 1) * NC_RAYS].T.astype(np.float32))
        for c in range(N_CORES)
    ]
    return featT_shards, coefT, labels_p


def kernel(origins, directions, means, covariances, labels_embedding):
    global LAST_RESULTS
    origins = np.asarray(origins, dtype=np.float32)
    directions = np.asarray(directions, dtype=np.float32)
    means = np.asarray(means, dtype=np.float32)
    covariances = np.asarray(covariances, dtype=np.float32)
    labels_embedding = np.asarray(labels_embedding, dtype=np.float32)
    if "nc" not in _CACHE:
        _CACHE["nc"] = _build_bass()
    nc = _CACHE["nc"]

    featT_shards, coefT, labels_p = _host_prepare(
        origins, directions, means, covariances, labels_embedding
    )
    in_maps = [
        {"featT": featT_shards[c], "coefT": coefT, "labels_p": labels_p}
        for c in range(N_CORES)
    ]
    res = bass_utils.run_bass_kernel_spmd(nc, in_maps, list(range(N_CORES)))
    LAST_RESULTS = res
    out = np.concatenate(
        [res.results[c]["out"].reshape(NC_RAYS, 1) for c in range(N_CORES)], axis=0
    )
    return out.astype(np.float32)
